# revision 17
# baseline (speedup 1.0000x reference)
"""Trainium2 Bass kernel for the CurvedAssociativeMemory fixed-point iteration.

Computes, for `steps` iterations:
    s <- sign(s @ (J + J^T) + h + kappa * softmax(s, axis=-1))

Strategy: data-parallel over the batch dim across 8 NeuronCores (512 rows
per core), J replicated and streamed from HBM each step.

The matmuls run in float32r (fp32 with an 11-bit stored mantissa) which the
PE processes at ~4x the fp32 rate.  Full fp32 precision is recovered with a
hi/lo split: J = Jh + Jl where Jh = fp32r(J), Jl = fp32r(J - Jh), giving
~23 mantissa bits across the pair.  The sign() applied each step makes the
iteration chaotic, so per-step precision is budgeted from a measured
noise-amplification study (flips in the final output per unit of noise
injected at step t):
  step 1 (real-valued s): 3 passes  sh@Jh + sl@Jh + sh@Jl   (eps ~2e-7)
  middle steps (s = +-1): 2 passes  s@Jh + s@Jl             (eps ~2e-7)
  final step:             1 pass    s@J16 (fp16)            (eps ~2.5e-4, OK
                                    because no further sign() amplifies it)
For step 1 the hi/lo split of s is precomputed on the host in transposed
layout (sT_hi persistent in SBUF, sT_lo streamed), so no on-chip
transposes are needed before the first matmuls.  Later steps transpose the
+-1 state on the PE; state tiles are float32r throughout so the PE
transposes run single-pass (1.5 cyc/row) instead of the 2-pass fp32
LOW_HIGH mode.  The final step streams J16 = fp16(64*Jsym) (half the HBM
traffic of fp32r; the measured DMA ceiling ~300GB/s was throttling the
final pass) against the float32r +-1 stationary state; the 64x scale keeps
J entries out of the fp16 subnormal range and is folded into the softmax
term's scale (sign(m + 64q) == sign(m/64 + q)).

The softmax term is computed without exp for +-1 states: softmax is
shift-invariant, so with n+ = (4096 + rowsum)/2,
  denom = n+ + (4096-n+)e^-2   and   e^{s_j-1} = alpha + beta*s_j
with alpha=(1+e^-2)/2, beta=(1-e^-2)/2 -- an affine function of the state,
evaluated as one fused tensor_scalar op per chunk.  Step 1 (real s) uses a
plain exp with a fused accumulated row-sum (no max subtraction needed:
|s| < ~5.5 cannot overflow, and softmax's shift invariance makes the
result identical to the reference's max-subtracted form).
"""

import numpy as np

N = 4096          # feature dim
B = 4096          # total batch
N_CORES = 8
B_SH = B // N_CORES   # 512 batch rows per core
P = 128               # partitions
NCHUNK = 512          # matmul moving free-dim per chunk
KO = N // P           # 32 k-tiles
NO = N // NCHUNK      # 8 n-chunks
BT = B_SH // P        # 4 batch tiles per core

# tuning knobs
JH_BUFS = 8      # Jh-tile prefetch depth
JL_BUFS = 7      # Jl-tile prefetch depth
J16_BUFS = 5     # fp16 J-tile prefetch depth (final step)
SLPOOL_BUFS = 3  # sT_lo ring depth (step 1 only)
PB_BUFS = 6      # matmul-accumulate PSUM tiles
PT_BUFS = 2      # transpose PSUM tiles (float32r)

FP16_FINAL = True   # stream fp16(64*J) for the final pass (mixed-dtype matmul)
F16_SCALE = 64.0

ALPHA = float((1.0 + np.exp(-2.0)) / 2.0)   # e^{c-1} = ALPHA + BETA*c for c=+-1
BETA = float((1.0 - np.exp(-2.0)) / 2.0)


def fp32r_round(x):
    """Round fp32 ndarray to fp32r: RNE to 11 stored mantissa bits (top 20
    bits of the fp32 word), matching the hardware/compiler convention."""
    u = np.ascontiguousarray(x, dtype=np.float32).view(np.uint32).astype(np.uint64)
    lsb = (u >> 12) & 1
    u = (u + 0x7FF + lsb) & np.uint64(0xFFFFF000)
    return u.astype(np.uint32).view(np.float32)


def _build(steps: int, kappa: float, has_h: bool):
    import concourse.bass as bass
    import concourse.tile as tile
    import concourse.mybir as mybir
    from concourse import bacc
    from concourse.masks import make_identity

    F32 = mybir.dt.float32
    F32R = mybir.dt.float32r
    F16 = mybir.dt.float16
    AF = mybir.ActivationFunctionType
    ALU = mybir.AluOpType

    use_f16 = FP16_FINAL and steps >= 2

    nc = bacc.Bacc(None)
    s_in = nc.dram_tensor("s", [B_SH, N], F32R, kind="ExternalInput")
    sth_in = nc.dram_tensor("sTh", [N, B_SH], F32R, kind="ExternalInput")
    stl_in = nc.dram_tensor("sTl", [N, B_SH], F32R, kind="ExternalInput")
    jh_in = nc.dram_tensor("Jh", [N, N], F32R, kind="ExternalInput")
    jl_in = nc.dram_tensor("Jl", [N, N], F32R, kind="ExternalInput")
    j16_in = (nc.dram_tensor("J16", [N, N], F16, kind="ExternalInput")
              if use_f16 else None)
    h_in = nc.dram_tensor("h", [N], F32, kind="ExternalInput") if has_h else None
    out = nc.dram_tensor("out", [B_SH, N], F32, kind="ExternalOutput")

    with tile.TileContext(nc) as tc:
        with (
            tc.tile_pool(name="persist", bufs=1) as persist,
            tc.tile_pool(name="jhpool", bufs=JH_BUFS) as jhpool,
            tc.tile_pool(name="jlpool", bufs=JL_BUFS) as jlpool,
            tc.tile_pool(name="j16pool", bufs=J16_BUFS) as j16pool,
            tc.tile_pool(name="slpool", bufs=SLPOOL_BUFS) as slpool,
            tc.tile_pool(name="scratch", bufs=2) as scratch,
            tc.tile_pool(name="stats", bufs=1) as stats,
            tc.tile_pool(name="psum", bufs=PB_BUFS, space="PSUM") as psum,
            tc.tile_pool(name="ptpsum", bufs=PT_BUFS, space="PSUM") as ptpsum,
        ):
            # identity for PE transposes, in fp32r so the transpose runs in
            # single-pass HIGH mode
            ident32 = persist.tile([P, P], F32, tag="ident32", name="ident32")
            make_identity(nc, ident32)
            ident = persist.tile([P, P], F32R, tag="ident", name="ident")
            nc.vector.tensor_copy(out=ident, in_=ident32)

            # Initial loads go on the Scalar (ACT) DMA queue so the Sync
            # queue can start issuing the J stream immediately.
            # transposed state (fp32r): loaded from sT_hi for step 1, then
            # regenerated by PE transposes each later step
            cTh = [persist.tile([P, B_SH], F32R, tag=f"th{k}", name=f"th{k}")
                   for k in range(KO)]
            for k in range(KO):
                nc.scalar.dma_start(out=cTh[k],
                                    in_=sth_in.ap()[k * P:(k + 1) * P, :])
            # fp16 copy of the transposed +-1 state, stationary operand for
            # the fp16 final pass (the PE cannot mix 32-bit and 16-bit
            # matmul inputs, so the final pass is fp16 x fp16)
            cTf = ([persist.tile([P, B_SH], F16, tag=f"tf{k}", name=f"tf{k}")
                    for k in range(KO)] if use_f16 else None)

            # persistent state in natural layout, 4 tiles of [128, N].
            # Declared fp32r (the DMA moves raw fp32 bytes unrounded; the PE
            # only ever transposes it once it is +-1) so the PE transposes
            # take the fast path.  DVE/ACT read it as plain fp32.
            c = [persist.tile([P, N], F32R, tag=f"c{bt}", name=f"c{bt}")
                 for bt in range(BT)]
            for bt in range(BT):
                nc.gpsimd.dma_start(out=c[bt], in_=s_in.ap()[bt * P:(bt + 1) * P, :])

            h_bc = None
            if has_h:
                h_bc = persist.tile([P, N], F32, tag="hb", name="hb")
                h_ap = h_in.ap()
                nc.sync.dma_start(
                    out=h_bc,
                    in_=bass.AP(tensor=h_ap.tensor, offset=h_ap.offset,
                                ap=[[0, P], [1, N]]),
                )

            rS = [stats.tile([P, 1], F32, tag=f"rS{bt}", name=f"rS{bt}")
                  for bt in range(BT)]   # step-1 epilogue scale kappa/ssum
            rSa = [stats.tile([P, 1], F32, tag=f"rSa{bt}", name=f"rSa{bt}")
                   for bt in range(BT)]  # keff*ALPHA/ssum
            rSb = [stats.tile([P, 1], F32, tag=f"rSb{bt}", name=f"rSb{bt}")
                   for bt in range(BT)]  # keff*BETA/ssum

            for t in range(steps):
                first = (t == 0)
                final = (t == steps - 1)
                # groups: (tag, J dram, pool, dtype, with_lo) -- with_lo adds
                # the sT_lo-stationary matmuls sharing the same streamed J tile
                if first:
                    groups = [("jh", jh_in, jhpool, F32R, True)]
                    if not final:
                        groups.append(("jl", jl_in, jlpool, F32R, False))
                elif final:
                    if use_f16:
                        groups = [("j16", j16_in, j16pool, F16, False)]
                    else:
                        groups = [("jh", jh_in, jhpool, F32R, False)]
                else:
                    groups = [("jh", jh_in, jhpool, F32R, False),
                              ("jl", jl_in, jlpool, F32R, False)]
                with_lo = groups[0][4]
                f16_scaled = final and not first and use_f16

                # transposes for this step's stationary were emitted inside the
                # previous step's last chunk (overlapped with its matmuls);
                # step 0 loads sTh from the host instead.
                keff = float(kappa) * (F16_SCALE if f16_scaled else 1.0)
                if first:
                    # real-valued s: ssum = sum(exp(s)) via chunked fused
                    # exp+row-accumulate (softmax is shift-invariant and
                    # |s| < ~5.5, so no max subtraction is needed)
                    for bt in range(BT):
                        acc = stats.tile([P, 1], F32, tag=f"ac{bt}",
                                         name=f"ac{bt}")
                        for ch in range(NO):
                            et = scratch.tile([P, NCHUNK], F32, tag="q", name="et")
                            pp = stats.tile([P, 1], F32, tag=f"pp{bt}_{ch % 2}",
                                            name=f"pp{bt}_{ch}")
                            nc.scalar.activation(
                                out=et, in_=c[bt][:, ch * NCHUNK:(ch + 1) * NCHUNK],
                                func=AF.Exp, accum_out=pp)
                            if ch == 0:
                                nc.vector.tensor_copy(out=acc, in_=pp)
                            else:
                                nc.vector.tensor_add(out=acc, in0=acc, in1=pp)
                        nc.vector.reciprocal(out=rS[bt], in_=acc)
                        nc.scalar.mul(out=rS[bt], in_=rS[bt], mul=keff)
                else:
                    # +-1 state: ssum = sum(exp(c-1)) is affine in the row sum
                    for bt in range(BT):
                        tr = stats.tile([P, 1], F32, tag=f"tr{bt}", name=f"tr{bt}")
                        nc.vector.reduce_sum(out=tr, in_=c[bt],
                                             axis=mybir.AxisListType.X)
                        sm = stats.tile([P, 1], F32, tag=f"sm{bt}", name=f"sm{bt}")
                        nc.vector.tensor_scalar(
                            out=sm, in0=tr, scalar1=BETA,
                            scalar2=float(2 * 2048 * ALPHA),
                            op0=ALU.mult, op1=ALU.add)
                        r0 = stats.tile([P, 1], F32, tag=f"r0{bt}", name=f"r0{bt}")
                        nc.vector.reciprocal(out=r0, in_=sm)
                        nc.scalar.mul(out=rSa[bt], in_=r0, mul=keff * ALPHA)
                        nc.scalar.mul(out=rSb[bt], in_=r0, mul=keff * BETA)

                # ---- phase B: matmul passes + epilogue per n-chunk ----
                # k-major: all J streams for a given k are loaded together so
                # the matmuls sharing the stationary cTh[k][bt] issue
                # back-to-back.  start/stop mark the per-PSUM-tile
                # accumulation sequence; every pm_t[bt] sees the same order.
                n_slots = KO * (len(groups) + (1 if with_lo else 0))
                cstat = cTf if f16_scaled else cTh
                for n in range(NO):
                    pm_t = [psum.tile([P, NCHUNK], F32, tag="pb", name="pm")
                            for _ in range(BT)]
                    slot = 0
                    for k in range(KO):
                        jts = []
                        for (jtag, j_dram, jpool, jdt, _wl) in groups:
                            jt = jpool.tile([P, NCHUNK], jdt, tag=jtag, name="jt")
                            nc.sync.dma_start(
                                out=jt,
                                in_=j_dram.ap()[k * P:(k + 1) * P,
                                                n * NCHUNK:(n + 1) * NCHUNK])
                            jts.append(jt)
                        if with_lo:
                            slt = slpool.tile([P, B_SH], F32R, tag="sl",
                                              name="slt")
                            nc.scalar.dma_start(
                                out=slt,
                                in_=stl_in.ap()[k * P:(k + 1) * P, :])
                        k_slots = len(jts) + (1 if with_lo else 0)
                        for bt in range(BT):
                            bsl = slice(bt * P, (bt + 1) * P)
                            sl_i = slot
                            for jt in jts:
                                nc.tensor.matmul(
                                    pm_t[bt], cstat[k][:, bsl], jt,
                                    start=(sl_i == 0),
                                    stop=(sl_i == n_slots - 1))
                                sl_i += 1
                            if with_lo:
                                nc.tensor.matmul(
                                    pm_t[bt], slt[:, bsl], jts[0],
                                    start=False,
                                    stop=(sl_i == n_slots - 1))
                                sl_i += 1
                        slot += k_slots
                        if (not final) and n == NO - 1 and k < (NO - 1) * BT:
                            # old cTh[k]/cTf[k] has no readers left; produce
                            # the next step's stationary for k now, overlapped
                            # with this chunk's remaining matmuls
                            tdst = cTf if (use_f16 and t + 1 == steps - 1) \
                                else cTh
                            pt = ptpsum.tile([P, NCHUNK], F32R, tag="pt",
                                             name="pt")
                            for bt in range(BT):
                                nc.tensor.transpose(
                                    pt[:, bt * P:(bt + 1) * P],
                                    c[bt][:, k * P:(k + 1) * P], ident)
                            nc.vector.tensor_copy(out=tdst[k], in_=pt)
                    nsl = slice(n * NCHUNK, (n + 1) * NCHUNK)
                    for bt in range(BT):
                        m_sl = pm_t[bt]
                        q = scratch.tile([P, NCHUNK], F32, tag="q", name="q")
                        if first:
                            nc.scalar.activation(out=q, in_=c[bt][:, nsl],
                                                 func=AF.Exp)
                            nc.vector.tensor_scalar_mul(out=q, in0=q,
                                                        scalar1=rS[bt])
                        else:
                            nc.vector.tensor_scalar(
                                out=q, in0=c[bt][:, nsl], scalar1=rSb[bt],
                                scalar2=rSa[bt], op0=ALU.mult, op1=ALU.add)
                        if has_h:
                            nc.vector.tensor_add(out=q, in0=q, in1=h_bc[:, nsl])
                        # accumulate the softmax term into PSUM in place; the
                        # sign then reads PSUM directly (no u scratch tile)
                        nc.vector.tensor_add(out=m_sl, in0=m_sl, in1=q)
                        if final:
                            # stream the final sign straight to DRAM per chunk
                            w = scratch.tile([P, NCHUNK], F32, tag="q", name="w")
                            nc.scalar.activation(out=w, in_=m_sl, func=AF.Sign)
                            nc.scalar.dma_start(
                                out=out.ap()[bt * P:(bt + 1) * P, nsl], in_=w)
                        else:
                            nc.scalar.activation(out=c[bt][:, nsl], in_=m_sl,
                                                 func=AF.Sign)
                    if (not final) and n == NO - 1:
                        tdst = cTf if (use_f16 and t + 1 == steps - 1) else cTh
                        for k in range((NO - 1) * BT, KO):
                            pt = ptpsum.tile([P, NCHUNK], F32R, tag="pt",
                                             name="pt")
                            for bt in range(BT):
                                nc.tensor.transpose(
                                    pt[:, bt * P:(bt + 1) * P],
                                    c[bt][:, k * P:(k + 1) * P], ident)
                            nc.vector.tensor_copy(out=tdst[k], in_=pt)

    nc.finalize()
    return nc


LAST_RESULTS = None  # BassKernelResults from the most recent kernel() call
LAST_NC = None       # finalized Bass module from the most recent kernel() call


def kernel(s, J, h, kappa, steps):
    import os
    from concourse.bass_utils import run_bass_kernel_spmd

    s = np.ascontiguousarray(np.asarray(s, dtype=np.float32))
    J = np.asarray(J, dtype=np.float32)
    h = np.asarray(h, dtype=np.float32)
    kappa_f = float(np.asarray(kappa))
    steps_i = int(np.asarray(steps))

    Jsym = np.ascontiguousarray(J + J.T)
    Jh = fp32r_round(Jsym)
    Jl = np.ascontiguousarray(fp32r_round(Jsym - Jh))
    Jh = np.ascontiguousarray(Jh)
    use_f16 = FP16_FINAL and steps_i >= 2
    J16 = (np.ascontiguousarray((Jsym * F16_SCALE).astype(np.float16))
           if use_f16 else None)
    has_h = bool(np.any(h))

    nc = _build(steps_i, kappa_f, has_h)
    global LAST_NC
    LAST_NC = nc

    in_maps = []
    for i in range(N_CORES):
        s_sh = np.ascontiguousarray(s[i * B_SH:(i + 1) * B_SH])
        sh = fp32r_round(s_sh)
        sl = fp32r_round(s_sh - sh)
        m = {"s": s_sh,
             "sTh": np.ascontiguousarray(sh.T),
             "sTl": np.ascontiguousarray(sl.T),
             "Jh": Jh, "Jl": Jl}
        if use_f16:
            m["J16"] = J16
        if has_h:
            m["h"] = h
        in_maps.append(m)

    trace = os.environ.get("CAM_TRACE", "") == "1"
    res = run_bass_kernel_spmd(nc, in_maps, core_ids=list(range(N_CORES)),
                               trace=trace)
    global LAST_RESULTS
    LAST_RESULTS = res
    out = np.concatenate([r["out"] for r in res.results], axis=0)
    return out.astype(np.float32, copy=False)


if __name__ == "__main__":
    rng = np.random.default_rng(0)
    s = rng.standard_normal((B, N)).astype(np.float32)
    J0 = (0.01 * rng.standard_normal((N, N))).astype(np.float32)
    J = ((J0 + J0.T) / 2).astype(np.float32)
    out = kernel(s=s, J=J, h=np.zeros(N, np.float32),
                 kappa=np.float32(0.2), steps=3)
    print(out.shape, np.unique(out, return_counts=True))


# revision 18
# speedup vs baseline: 1.0415x; 1.0415x over previous
"""Trainium2 Bass kernel for the CurvedAssociativeMemory fixed-point iteration.

Computes, for `steps` iterations:
    s <- sign(s @ (J + J^T) + h + kappa * softmax(s, axis=-1))

Strategy: data-parallel over the batch dim across 8 NeuronCores (512 rows
per core), J replicated and streamed from HBM each step.

The matmuls run in float32r (fp32 with an 11-bit stored mantissa) which the
PE processes at ~4x the fp32 rate.  Full fp32 precision is recovered with a
hi/lo split: J = Jh + Jl where Jh = fp32r(J), Jl = fp32r(J - Jh), giving
~23 mantissa bits across the pair.  The sign() applied each step makes the
iteration chaotic, so per-step precision is budgeted from a measured
noise-amplification study (flips in the final output per unit of noise
injected at step t):
  step 1 (real-valued s): 3 passes  sh@Jh + sl@Jh + sh@Jl   (eps ~2e-7)
  middle steps (s = +-1): 2 passes  s@Jh + s@Jl             (eps ~2e-7)
  final step:             1 pass    s@J16 (fp16)            (eps ~2.5e-4, OK
                                    because no further sign() amplifies it)
For step 1 the hi/lo split of s is precomputed on the host in transposed
layout (sT_hi persistent in SBUF, sT_lo streamed), so no on-chip
transposes are needed before the first matmuls.  Later steps transpose the
+-1 state on the PE; state tiles are float32r throughout so the PE
transposes run single-pass (1.5 cyc/row) instead of the 2-pass fp32
LOW_HIGH mode.  The final step streams J16 = fp16(64*Jsym) (half the HBM
traffic of fp32r; the measured DMA ceiling ~300GB/s was throttling the
final pass) against the float32r +-1 stationary state; the 64x scale keeps
J entries out of the fp16 subnormal range and is folded into the softmax
term's scale (sign(m + 64q) == sign(m/64 + q)).

The softmax term is computed without exp for +-1 states: softmax is
shift-invariant, so with n+ = (4096 + rowsum)/2,
  denom = n+ + (4096-n+)e^-2   and   e^{s_j-1} = alpha + beta*s_j
with alpha=(1+e^-2)/2, beta=(1-e^-2)/2 -- an affine function of the state,
evaluated as one fused tensor_scalar op per chunk.  Step 1 (real s) uses a
plain exp with a fused accumulated row-sum (no max subtraction needed:
|s| < ~5.5 cannot overflow, and softmax's shift invariance makes the
result identical to the reference's max-subtracted form).
"""

import numpy as np

N = 4096          # feature dim
B = 4096          # total batch
N_CORES = 8
B_SH = B // N_CORES   # 512 batch rows per core
P = 128               # partitions
NCHUNK = 512          # matmul moving free-dim per chunk
KO = N // P           # 32 k-tiles
NO = N // NCHUNK      # 8 n-chunks
BT = B_SH // P        # 4 batch tiles per core

# tuning knobs
JH_BUFS = 8      # Jh-tile prefetch depth
JL_BUFS = 6      # Jl-tile prefetch depth
J16_BUFS = 6     # fp16 J-tile prefetch depth (final step)
SLPOOL_BUFS = 3  # sT_lo ring depth (step 1 only)
PB_BUFS = 7      # matmul-accumulate PSUM tiles
PT_BUFS = 1      # transpose PSUM tiles (float32r)

FP16_FINAL = True   # stream fp16(64*J) for the final pass (mixed-dtype matmul)
F16_SCALE = 64.0

ALPHA = float((1.0 + np.exp(-2.0)) / 2.0)   # e^{c-1} = ALPHA + BETA*c for c=+-1
BETA = float((1.0 - np.exp(-2.0)) / 2.0)


def fp32r_round(x):
    """Round fp32 ndarray to fp32r: RNE to 11 stored mantissa bits (top 20
    bits of the fp32 word), matching the hardware/compiler convention."""
    u = np.ascontiguousarray(x, dtype=np.float32).view(np.uint32).astype(np.uint64)
    lsb = (u >> 12) & 1
    u = (u + 0x7FF + lsb) & np.uint64(0xFFFFF000)
    return u.astype(np.uint32).view(np.float32)


def _build(steps: int, kappa: float, has_h: bool):
    import concourse.bass as bass
    import concourse.tile as tile
    import concourse.mybir as mybir
    from concourse import bacc
    from concourse.masks import make_identity

    F32 = mybir.dt.float32
    F32R = mybir.dt.float32r
    F16 = mybir.dt.float16
    AF = mybir.ActivationFunctionType
    ALU = mybir.AluOpType

    use_f16 = FP16_FINAL and steps >= 2

    nc = bacc.Bacc(None)
    s_in = nc.dram_tensor("s", [B_SH, N], F32R, kind="ExternalInput")
    sth_in = nc.dram_tensor("sTh", [N, B_SH], F32R, kind="ExternalInput")
    stl_in = nc.dram_tensor("sTl", [N, B_SH], F32R, kind="ExternalInput")
    jh_in = nc.dram_tensor("Jh", [N, N], F32R, kind="ExternalInput")
    jl_in = nc.dram_tensor("Jl", [N, N], F32R, kind="ExternalInput")
    j16_in = (nc.dram_tensor("J16", [N, N], F16, kind="ExternalInput")
              if use_f16 else None)
    h_in = nc.dram_tensor("h", [N], F32, kind="ExternalInput") if has_h else None
    out = nc.dram_tensor("out", [B_SH, N], F32, kind="ExternalOutput")

    with tile.TileContext(nc) as tc:
        with (
            tc.tile_pool(name="persist", bufs=1) as persist,
            tc.tile_pool(name="jhpool", bufs=JH_BUFS) as jhpool,
            tc.tile_pool(name="jlpool", bufs=JL_BUFS) as jlpool,
            tc.tile_pool(name="j16pool", bufs=J16_BUFS) as j16pool,
            tc.tile_pool(name="slpool", bufs=SLPOOL_BUFS) as slpool,
            tc.tile_pool(name="scratch", bufs=2) as scratch,
            tc.tile_pool(name="stats", bufs=1) as stats,
            tc.tile_pool(name="psum", bufs=PB_BUFS, space="PSUM") as psum,
            tc.tile_pool(name="ptpsum", bufs=PT_BUFS, space="PSUM") as ptpsum,
        ):
            # identity for PE transposes, in fp32r so the transpose runs in
            # single-pass HIGH mode
            ident32 = persist.tile([P, P], F32, tag="ident32", name="ident32")
            make_identity(nc, ident32)
            ident = persist.tile([P, P], F32R, tag="ident", name="ident")
            nc.vector.tensor_copy(out=ident, in_=ident32)

            # Initial loads go on the Scalar (ACT) DMA queue so the Sync
            # queue can start issuing the J stream immediately.
            # transposed state (fp32r): loaded from sT_hi for step 1, then
            # regenerated by PE transposes each later step
            cTh = [persist.tile([P, B_SH], F32R, tag=f"th{k}", name=f"th{k}")
                   for k in range(KO)]
            for k in range(KO):
                nc.scalar.dma_start(out=cTh[k],
                                    in_=sth_in.ap()[k * P:(k + 1) * P, :])
            # fp16 copy of the transposed +-1 state, stationary operand for
            # the fp16 final pass (the PE cannot mix 32-bit and 16-bit
            # matmul inputs, so the final pass is fp16 x fp16)
            cTf = ([persist.tile([P, B_SH], F16, tag=f"tf{k}", name=f"tf{k}")
                    for k in range(KO)] if use_f16 else None)

            # persistent state in natural layout, 4 tiles of [128, N].
            # Declared fp32r (the DMA moves raw fp32 bytes unrounded; the PE
            # only ever transposes it once it is +-1) so the PE transposes
            # take the fast path.  DVE/ACT read it as plain fp32.
            # The loads are emitted inside step 0's first chunk (below) so
            # the scalar ring delivers sTh and the chunk-0 sT_lo tiles
            # first; c is only needed by the chunk-0 epilogue, which the
            # PSUM ring lets lag well into chunk 1.
            c = [persist.tile([P, N], F32R, tag=f"c{bt}", name=f"c{bt}")
                 for bt in range(BT)]

            h_bc = None
            if has_h:
                h_bc = persist.tile([P, N], F32, tag="hb", name="hb")
                h_ap = h_in.ap()
                nc.sync.dma_start(
                    out=h_bc,
                    in_=bass.AP(tensor=h_ap.tensor, offset=h_ap.offset,
                                ap=[[0, P], [1, N]]),
                )

            rS = [stats.tile([P, 1], F32, tag=f"rS{bt}", name=f"rS{bt}")
                  for bt in range(BT)]   # step-1 epilogue scale kappa/ssum
            rSa = [stats.tile([P, 1], F32, tag=f"rSa{bt}", name=f"rSa{bt}")
                   for bt in range(BT)]  # keff*ALPHA/ssum
            rSb = [stats.tile([P, 1], F32, tag=f"rSb{bt}", name=f"rSb{bt}")
                   for bt in range(BT)]  # keff*BETA/ssum

            for t in range(steps):
                first = (t == 0)
                final = (t == steps - 1)
                # groups: (tag, J dram, pool, dtype, with_lo) -- with_lo adds
                # the sT_lo-stationary matmuls sharing the same streamed J tile
                if first:
                    groups = [("jh", jh_in, jhpool, F32R, True)]
                    if not final:
                        groups.append(("jl", jl_in, jlpool, F32R, False))
                elif final:
                    if use_f16:
                        groups = [("j16", j16_in, j16pool, F16, False)]
                    else:
                        groups = [("jh", jh_in, jhpool, F32R, False)]
                else:
                    groups = [("jh", jh_in, jhpool, F32R, False),
                              ("jl", jl_in, jlpool, F32R, False)]
                with_lo = groups[0][4]
                f16_scaled = final and not first and use_f16

                # transposes for this step's stationary were emitted inside the
                # previous step's last chunk (overlapped with its matmuls);
                # step 0 loads sTh from the host instead.
                keff = float(kappa) * (F16_SCALE if f16_scaled else 1.0)

                def emit_first_stats():
                    # real-valued s: ssum = sum(exp(s)) via chunked fused
                    # exp+row-accumulate (softmax is shift-invariant and
                    # |s| < ~5.5, so no max subtraction is needed)
                    for bt in range(BT):
                        nc.scalar.dma_start(
                            out=c[bt], in_=s_in.ap()[bt * P:(bt + 1) * P, :])
                    for bt in range(BT):
                        acc = stats.tile([P, 1], F32, tag=f"ac{bt}",
                                         name=f"ac{bt}")
                        for ch in range(NO):
                            et = scratch.tile([P, NCHUNK], F32, tag="q", name="et")
                            pp = stats.tile([P, 1], F32, tag=f"pp{bt}_{ch % 2}",
                                            name=f"pp{bt}_{ch}")
                            nc.scalar.activation(
                                out=et, in_=c[bt][:, ch * NCHUNK:(ch + 1) * NCHUNK],
                                func=AF.Exp, accum_out=pp)
                            if ch == 0:
                                nc.vector.tensor_copy(out=acc, in_=pp)
                            else:
                                nc.vector.tensor_add(out=acc, in0=acc, in1=pp)
                        nc.vector.reciprocal(out=rS[bt], in_=acc)
                        nc.scalar.mul(out=rS[bt], in_=rS[bt], mul=keff)

                if first:
                    pass  # stats emitted inside chunk 0 (emit_first_stats)
                else:
                    # +-1 state: ssum = sum(exp(c-1)) is affine in the row sum
                    for bt in range(BT):
                        tr = stats.tile([P, 1], F32, tag=f"tr{bt}", name=f"tr{bt}")
                        nc.vector.reduce_sum(out=tr, in_=c[bt],
                                             axis=mybir.AxisListType.X)
                        sm = stats.tile([P, 1], F32, tag=f"sm{bt}", name=f"sm{bt}")
                        nc.vector.tensor_scalar(
                            out=sm, in0=tr, scalar1=BETA,
                            scalar2=float(2 * 2048 * ALPHA),
                            op0=ALU.mult, op1=ALU.add)
                        r0 = stats.tile([P, 1], F32, tag=f"r0{bt}", name=f"r0{bt}")
                        nc.vector.reciprocal(out=r0, in_=sm)
                        nc.scalar.mul(out=rSa[bt], in_=r0, mul=keff * ALPHA)
                        nc.scalar.mul(out=rSb[bt], in_=r0, mul=keff * BETA)

                # ---- phase B: matmul passes + epilogue per n-chunk ----
                # k-major: all J streams for a given k are loaded together so
                # the matmuls sharing the stationary cTh[k][bt] issue
                # back-to-back.  start/stop mark the per-PSUM-tile
                # accumulation sequence; every pm_t[bt] sees the same order.
                n_slots = KO * (len(groups) + (1 if with_lo else 0))
                cstat = cTf if f16_scaled else cTh
                for n in range(NO):
                    pm_t = [psum.tile([P, NCHUNK], F32, tag="pb", name="pm")
                            for _ in range(BT)]
                    slot = 0
                    for k in range(KO):
                        jts = []
                        for (jtag, j_dram, jpool, jdt, _wl) in groups:
                            jt = jpool.tile([P, NCHUNK], jdt, tag=jtag, name="jt")
                            nc.sync.dma_start(
                                out=jt,
                                in_=j_dram.ap()[k * P:(k + 1) * P,
                                                n * NCHUNK:(n + 1) * NCHUNK])
                            jts.append(jt)
                        if with_lo:
                            slt = slpool.tile([P, B_SH], F32R, tag="sl",
                                              name="slt")
                            nc.scalar.dma_start(
                                out=slt,
                                in_=stl_in.ap()[k * P:(k + 1) * P, :])
                        k_slots = len(jts) + (1 if with_lo else 0)
                        for bt in range(BT):
                            bsl = slice(bt * P, (bt + 1) * P)
                            sl_i = slot
                            for jt in jts:
                                nc.tensor.matmul(
                                    pm_t[bt], cstat[k][:, bsl], jt,
                                    start=(sl_i == 0),
                                    stop=(sl_i == n_slots - 1))
                                sl_i += 1
                            if with_lo:
                                nc.tensor.matmul(
                                    pm_t[bt], slt[:, bsl], jts[0],
                                    start=False,
                                    stop=(sl_i == n_slots - 1))
                                sl_i += 1
                        slot += k_slots
                        if first and n == 0 and k == KO - 1:
                            emit_first_stats()
                        if (not final) and n == NO - 1 and k < (NO - 1) * BT:
                            # old cTh[k]/cTf[k] has no readers left; produce
                            # the next step's stationary for k now, overlapped
                            # with this chunk's remaining matmuls
                            tdst = cTf if (use_f16 and t + 1 == steps - 1) \
                                else cTh
                            pt = ptpsum.tile([P, NCHUNK], F32R, tag="pt",
                                             name="pt")
                            for bt in range(BT):
                                nc.tensor.transpose(
                                    pt[:, bt * P:(bt + 1) * P],
                                    c[bt][:, k * P:(k + 1) * P], ident)
                            nc.vector.tensor_copy(out=tdst[k], in_=pt)
                    nsl = slice(n * NCHUNK, (n + 1) * NCHUNK)
                    for bt in range(BT):
                        m_sl = pm_t[bt]
                        q = scratch.tile([P, NCHUNK], F32, tag="q", name="q")
                        if first:
                            nc.scalar.activation(out=q, in_=c[bt][:, nsl],
                                                 func=AF.Exp)
                            nc.vector.tensor_scalar_mul(out=q, in0=q,
                                                        scalar1=rS[bt])
                        else:
                            nc.vector.tensor_scalar(
                                out=q, in0=c[bt][:, nsl], scalar1=rSb[bt],
                                scalar2=rSa[bt], op0=ALU.mult, op1=ALU.add)
                        if has_h:
                            nc.vector.tensor_add(out=q, in0=q, in1=h_bc[:, nsl])
                        # accumulate the softmax term into PSUM in place; the
                        # sign then reads PSUM directly (no u scratch tile)
                        nc.vector.tensor_add(out=m_sl, in0=m_sl, in1=q)
                        if final:
                            # stream the final sign straight to DRAM per chunk
                            w = scratch.tile([P, NCHUNK], F32, tag="q", name="w")
                            nc.scalar.activation(out=w, in_=m_sl, func=AF.Sign)
                            nc.scalar.dma_start(
                                out=out.ap()[bt * P:(bt + 1) * P, nsl], in_=w)
                        else:
                            nc.scalar.activation(out=c[bt][:, nsl], in_=m_sl,
                                                 func=AF.Sign)
                    if (not final) and n == NO - 1:
                        tdst = cTf if (use_f16 and t + 1 == steps - 1) else cTh
                        for k in range((NO - 1) * BT, KO):
                            pt = ptpsum.tile([P, NCHUNK], F32R, tag="pt",
                                             name="pt")
                            for bt in range(BT):
                                nc.tensor.transpose(
                                    pt[:, bt * P:(bt + 1) * P],
                                    c[bt][:, k * P:(k + 1) * P], ident)
                            nc.vector.tensor_copy(out=tdst[k], in_=pt)

    nc.finalize()
    return nc


LAST_RESULTS = None  # BassKernelResults from the most recent kernel() call
LAST_NC = None       # finalized Bass module from the most recent kernel() call


def kernel(s, J, h, kappa, steps):
    import os
    from concourse.bass_utils import run_bass_kernel_spmd

    s = np.ascontiguousarray(np.asarray(s, dtype=np.float32))
    J = np.asarray(J, dtype=np.float32)
    h = np.asarray(h, dtype=np.float32)
    kappa_f = float(np.asarray(kappa))
    steps_i = int(np.asarray(steps))

    Jsym = np.ascontiguousarray(J + J.T)
    Jh = fp32r_round(Jsym)
    Jl = np.ascontiguousarray(fp32r_round(Jsym - Jh))
    Jh = np.ascontiguousarray(Jh)
    use_f16 = FP16_FINAL and steps_i >= 2
    J16 = (np.ascontiguousarray((Jsym * F16_SCALE).astype(np.float16))
           if use_f16 else None)
    has_h = bool(np.any(h))

    nc = _build(steps_i, kappa_f, has_h)
    global LAST_NC
    LAST_NC = nc

    in_maps = []
    for i in range(N_CORES):
        s_sh = np.ascontiguousarray(s[i * B_SH:(i + 1) * B_SH])
        sh = fp32r_round(s_sh)
        sl = fp32r_round(s_sh - sh)
        m = {"s": s_sh,
             "sTh": np.ascontiguousarray(sh.T),
             "sTl": np.ascontiguousarray(sl.T),
             "Jh": Jh, "Jl": Jl}
        if use_f16:
            m["J16"] = J16
        if has_h:
            m["h"] = h
        in_maps.append(m)

    trace = os.environ.get("CAM_TRACE", "") == "1"
    res = run_bass_kernel_spmd(nc, in_maps, core_ids=list(range(N_CORES)),
                               trace=trace)
    global LAST_RESULTS
    LAST_RESULTS = res
    out = np.concatenate([r["out"] for r in res.results], axis=0)
    return out.astype(np.float32, copy=False)


if __name__ == "__main__":
    rng = np.random.default_rng(0)
    s = rng.standard_normal((B, N)).astype(np.float32)
    J0 = (0.01 * rng.standard_normal((N, N))).astype(np.float32)
    J = ((J0 + J0.T) / 2).astype(np.float32)
    out = kernel(s=s, J=J, h=np.zeros(N, np.float32),
                 kappa=np.float32(0.2), steps=3)
    print(out.shape, np.unique(out, return_counts=True))
